# revision 25
# baseline (speedup 1.0000x reference)
"""DenseQConv1D Trainium2 kernel.

Math: the reference computes, per output channel c and patch p (128-dim im2col
column of x, normalized):
    out[c,p] = sum_e sign(e) * (s_p^T (E @ R_c)[:128,:])_e^2
with R_c = kron of 9 RY(theta[c,q]) rotations and sign(e) = Z on the MSB qubit.
Because every RY factor is orthogonal and the measurement only touches qubit 0,
    R_c S R_c^T = kron([[cos t, sin t], [sin t, -cos t]], I_256),  t = theta[c,0]
so with E128 = E[:128,:], F = E128[:,:256], G = E128[:,256:]:
    GZ = F F^T - G G^T,  GX = F G^T + G F^T   (both 128x128, theta-independent)
    out[c,p] = (cos t_c * p^T GZ p + sin t_c * p^T GX p) / ||p||^2

Fast path: the entangle matrix built by the reference is a composition of CNOT
permutations, i.e. a permutation matrix that is linear over GF(2).  For any
such E, GZ is a +/-1 diagonal and GX restricted to the 128-dim patch subspace
is either an XOR-pairing or zero; for the CNOT ring used here GX == 0.  Then
    out[c,l] = cos t_c * (sum_j d_j pt[j,l]^2) / (sum_j pt[j,l]^2)
which needs no dense quadratic form at all.  The host folds theta and E into a
single [128, 48] stationary W = [d_j*cos_c | 0 | ones] and the device computes,
per l-chunk: sq = pt*pt (DVE, bf16), ps = W^T @ sq (PE: rows 0:16 give
cos_c*z_l, rows 32:48 give n2_l broadcast 16-wide), invb = 1/ps[32:48] (ACT,
which also bridges the 32-aligned PSUM partitions down to 0:16), and
out = ps[0:16]*invb (DVE).  The host verifies the GZ/GX structure numerically
and falls back to the fully generic kernel below for any other entangle input.

Sharding: batch dimension across the 8 cores (core b computes x[b]).
"""

import math
from contextlib import ExitStack as _ExitStack

import numpy as np

B = 8
C_IN = 16
C_OUT = 16
L = 1024
K = 8
L_OUT = L - K + 1  # 1017
LP = 1024  # padded patch count per core (cols 1017:1024 are dummy)
P = 128  # patch vector length = C_IN*K = partitions
XPAD = L + K  # padded x row length so every im2col window read is in-bounds

_CACHE = {}


def _build_nc_fast():
    import bass_rust as _br
    import concourse.bacc as bacc
    import concourse.mybir as mybir
    import concourse.tile as tile

    f32 = mybir.dt.float32
    bf16 = mybir.dt.bfloat16
    AF = mybir.ActivationFunctionType

    nc = bacc.Bacc("TRN2", target_bir_lowering=False, debug=False)

    def act_raw(out, in_, func, bias=0.0, scale=1.0):
        eng = nc.scalar
        ins = [
            eng.lower_ap(in_),
            mybir.ImmediateValue(dtype=mybir.dt.float32, value=bias),
            mybir.ImmediateValue(dtype=mybir.dt.float32, value=scale),
            mybir.ImmediateValue(dtype=mybir.dt.float32, value=0.0),
        ]
        return eng.add_instruction(
            mybir.InstActivation(
                name=nc.get_next_instruction_name(), func=func,
                ins=ins, outs=[eng.lower_ap(out)],
            )
        )

    xb_ext = nc.declare_dram_parameter("xb", [C_IN, XPAD], bf16, isOutput=False)
    wm_ext = nc.declare_dram_parameter("wm", [P, 3 * C_OUT], bf16, isOutput=False)
    out_ext = nc.declare_dram_parameter("out", [C_OUT, LP], f32, isOutput=True)

    with tile.TileContext(nc) as tc, tc.tile_pool(name="sb", bufs=1) as sb, \
            tc.tile_pool(name="ps", bufs=1, space="PSUM") as psp:
        # pt chunks on the sync ring first — they gate everything; uneven
        # chunks (big first, small last) so early chunks' compute overlaps
        # later transfers and the final chunk's serial tail is short
        # (measured best vs 2x512, 4x256, 1x1024, and dual-ring splits —
        # the first-chunk latency is HBM-latency-bound, and smaller chunks
        # pay extra ACTIVATE pipeline fills and DMA issue slots)
        chunks = [(0, 512), (512, 384), (896, 128)]
        pt = sb.tile([P, LP], bf16)
        for lo, n in chunks:
            apx = _br.AP(xb_ext, lo, [[1, K], [XPAD, C_IN], [1, n]])
            nc.sync.dma_start(pt[:, lo : lo + n], apx)

        # wm on the scalar ring, then the table-load trigger (first ACT op is
        # a Reciprocal so the single table set loads during the input DMA)
        wmt = sb.tile([P, 3 * C_OUT], bf16)
        nc.scalar.dma_start(wmt[:], wm_ext[:, :])
        tdum = sb.tile([1, 1], f32)
        nc.vector.memset(tdum[:], 1.0)
        tdum2 = sb.tile([1, 1], f32)
        act_raw(tdum2[:], tdum[:], AF.Reciprocal, bias=1e-24)

        sq = sb.tile([P, LP], bf16)
        # engine operand partition bases must be 32-aligned, so n2 lands at
        # PSUM partitions 32:48 (wm cols 32:48 are ones) and the ACT
        # reciprocal bridges it down to partitions 0:16 for the DVE multiply
        invb = sb.tile([C_OUT, LP], f32)
        outs = sb.tile([C_OUT, LP], f32)
        ph = []
        for i, (lo, n) in enumerate(chunks):
            s = slice(lo, lo + n)
            nc.vector.tensor_mul(sq[:, s], pt[:, s], pt[:, s])
            # full-bank tiles so chunks never share a PSUM bank
            ph.append(psp.tile([3 * C_OUT, 512], f32, name=f"ps{i}", tag=f"ps{i}"))
            nc.tensor.matmul(
                ph[i][:, 0:n], wmt[:], sq[:, s], start=True, stop=True
            )
        for i, (lo, n) in enumerate(chunks):
            s = slice(lo, lo + n)
            act_raw(
                invb[:, s], ph[i][2 * C_OUT : 3 * C_OUT, 0:n],
                AF.Reciprocal, bias=1e-24,
            )
        for i, (lo, n) in enumerate(chunks):
            s = slice(lo, lo + n)
            nc.vector.tensor_mul(outs[:, s], ph[i][0:C_OUT, 0:n], invb[:, s])
            # out chunks alternate across the two HWDGE rings
            eng = nc.sync if i % 2 == 0 else nc.scalar
            eng.dma_start(out_ext[:, s], outs[:, s])

    nc.compile()
    return nc


def _host_fold(x, theta, ent):
    """Fold theta and the entangle matrix into the fast-path inputs.

    Returns a list of per-core in_maps, or None if the entangle matrix does
    not have the (GZ diagonal, GX == 0) structure the fast kernel assumes.
    """
    import ml_dtypes

    E128 = ent[:P]
    F, G = E128[:, :256], E128[:, 256:]
    GZ = F @ F.T - G @ G.T
    GX = F @ G.T + G @ F.T
    dg = np.diag(GZ).copy()
    if np.abs(GZ - np.diag(dg)).max() > 1e-5 or np.abs(GX).max() > 1e-5:
        return None
    cos = np.cos(theta[:, 0].astype(np.float64))
    # device partition p = j*16+c maps to reference feature f = c*8+j
    p = np.arange(P)
    d_dev = dg[(p % C_IN) * K + p // C_IN]
    wm = np.zeros((P, 3 * C_OUT), np.float32)
    wm[:, :C_OUT] = d_dev[:, None] * cos[None, :]
    wm[:, 2 * C_OUT :] = 1.0
    wm_bf = np.ascontiguousarray(wm.astype(ml_dtypes.bfloat16))
    xpad = np.full((B, C_IN, XPAD), 1.0, np.float32)
    xpad[:, :, :L] = x
    xb = xpad.astype(ml_dtypes.bfloat16)
    return [
        {"xb": np.ascontiguousarray(xb[b]), "wm": wm_bf} for b in range(B)
    ]


def prepare(inputs):
    """Returns (nc, in_maps). Shared by kernel() and test.py."""
    x = np.ascontiguousarray(np.asarray(inputs["x"], dtype=np.float32))
    theta = np.ascontiguousarray(np.asarray(inputs["theta"], dtype=np.float32))
    ent = np.ascontiguousarray(
        np.asarray(inputs["entangle_matrix"], dtype=np.float32)
    )
    fold = _host_fold(x, theta, ent)
    if fold is not None:
        if "nc_fast" not in _CACHE:
            _CACHE["nc_fast"] = _build_nc_fast()
        return _CACHE["nc_fast"], fold
    if "nc" not in _CACHE:
        _CACHE["nc"] = _build_nc()
    in_maps = [
        {"x": np.ascontiguousarray(x[b]), "theta": theta, "entangle": ent}
        for b in range(B)
    ]
    return _CACHE["nc"], in_maps


def kernel(**inputs):
    from concourse.bass_utils import run_bass_kernel_spmd

    nc, in_maps = prepare(inputs)
    fast = nc is _CACHE.get("nc_fast")
    # fast path: |out[c,l]| = |cos(t_c) * z_l / n2_l| <= 1 mathematically
    # (|z| <= n2); anything far above that means a transient device glitch
    # -> retry (the bound needs E orthogonal, so fast path only)
    for _attempt in range(3):
        res = run_bass_kernel_spmd(nc, in_maps, core_ids=list(range(B)))
        out = np.stack(
            [res.results[b]["out"][:, :L_OUT] for b in range(B)], axis=0
        )
        ok = np.isfinite(out).all() and (not fast or np.abs(out).max() < 1.25)
        if ok:
            break
    return np.ascontiguousarray(out.astype(np.float32))


# ---------------------------------------------------------------------------
# Generic fallback: exact for any entangle matrix / theta (dense GZ/GX built
# on device; fp32r quadratic forms).  Used only when _host_fold rejects.
# ---------------------------------------------------------------------------

def _build_nc(dbg=False):
    import bass_rust as _br
    import concourse.bacc as bacc
    import concourse.mybir as mybir
    import concourse.tile as tile
    from concourse import masks

    f32 = mybir.dt.float32
    f32r = mybir.dt.float32r
    AF = mybir.ActivationFunctionType
    ALU = mybir.AluOpType

    def act_raw(out, in_, func, bias=0.0, scale=1.0):
        eng = nc.scalar
        ins = [
            eng.lower_ap(in_),
            mybir.ImmediateValue(dtype=mybir.dt.float32, value=bias),
            mybir.ImmediateValue(dtype=mybir.dt.float32, value=scale),
            mybir.ImmediateValue(dtype=mybir.dt.float32, value=0.0),
        ]
        return eng.add_instruction(
            mybir.InstActivation(
                name=nc.get_next_instruction_name(), func=func,
                ins=ins, outs=[eng.lower_ap(out)],
            )
        )
    nc = bacc.Bacc("TRN2", target_bir_lowering=False, debug=False)

    x_ext = nc.declare_dram_parameter("x", [C_IN, L], f32, isOutput=False)
    th_ext = nc.declare_dram_parameter("theta", [C_OUT, 9], f32, isOutput=False)
    e_ext = nc.declare_dram_parameter("entangle", [512, 512], f32, isOutput=False)
    out_ext = nc.declare_dram_parameter("out", [C_OUT, LP], f32, isOutput=True)

    with tile.TileContext(nc) as tc, tc.tile_pool(name="const", bufs=1) as const, \
            tc.tile_pool(name="sb", bufs=1) as sb, \
            tc.tile_pool(name="scr", bufs=2) as scrp:
        ident = const.tile([P, P], f32)
        masks.make_identity(nc, ident[:])
        ones1 = const.tile([1, P], f32)
        nc.vector.memset(ones1[:], 1.0)
        onesc = const.tile([P, 1], f32)
        nc.vector.memset(onesc[:], 1.0)

        # ---- loads: theta (tiny) first, then E halves, then x halves;
        # interleaved across the two HWDGE rings so E lands first ----
        th = sb.tile([1, C_OUT], f32)
        nc.sync.dma_start(th[:], th_ext[:, 0:1].rearrange("p o -> o p"))

        # E rows 0..127, row-permuted (c j) -> (j c); partition halves on the
        # two HWDGE rings (the gpsimd SWDGE path is ~26x slower here - avoid)
        e128 = sb.tile([P, 512], f32)
        e_p0 = _br.AP(e_ext, 0, [[512, 4], [512 * K, C_IN], [1, 512]])
        e_p1 = _br.AP(e_ext, 4 * 512, [[512, 4], [512 * K, C_IN], [1, 512]])
        nc.sync.dma_start(e128[0:64, :], e_p0)
        nc.scalar.dma_start(e128[64:P, :], e_p1)

        # PT[j*16+c, l] = x[c, l+j]: im2col via overlapping-window APs
        pt = sb.tile([P, LP], f32)
        # l-halves so the first 512 patch columns can be consumed while the
        # second half still streams; partition halves across the two rings
        for hl, (lo, ln) in enumerate([(0, 512), (512, L_OUT - 512)]):
            xw0 = _br.AP(x_ext, lo, [[1, 4], [L, C_IN], [1, ln]])
            xw1 = _br.AP(x_ext, 4 + lo, [[1, 4], [L, C_IN], [1, ln]])
            nc.sync.dma_start(pt[0:64, lo : lo + ln], xw0)
            nc.scalar.dma_start(pt[64:P, lo : lo + ln], xw1)
        nc.vector.memset(pt[:, L_OUT:LP], 1.0)

        # ACT uses exactly two PWP tables: trig_and_small (sin/square/copy)
        # then natural_log_exp_and_others (ln/exp/square/copy). Biases are DVE
        # memsets so the first ACT op is the Sin that loads the trig table;
        # a dummy Ln right after the trig block performs the single switch.
        bias_zero = const.tile([1, 1], f32)
        nc.vector.memset(bias_zero[:], 0.0)
        bias_half_pi = const.tile([1, 1], f32)
        nc.vector.memset(bias_half_pi[:], math.pi / 2.0)
        bias_eps = const.tile([1, 1], f32)
        nc.vector.memset(bias_eps[:], 1e-24)
        tdum = const.tile([1, 1], f32)

        # |t| and sign(t) on DVE (cheap, keeps ACT free)
        csrow = sb.tile([1, 2 * C_OUT], f32r)
        ta = sb.tile([1, C_OUT], f32)
        nc.vector.scalar_tensor_tensor(
            ta[:], th[:], -1.0, th[:], op0=ALU.mult, op1=ALU.max
        )
        tsgn = sb.tile([1, C_OUT], f32)
        nc.vector.tensor_scalar(tsgn[:], th[:], 0.0, None, op0=ALU.is_gt)
        nc.vector.tensor_scalar(
            tsgn[:], tsgn[:], 2.0, 1.0, op0=ALU.mult, op1=ALU.subtract
        )

        # trig: u = sin(|t|/2), v = cos(|t|/2); cos t = 1-2u^2,
        # sin t = sign(t)*2uv
        nc.scalar.activation(tdum[:], bias_zero[:], AF.Sin, bias=bias_zero[:])
        u = sb.tile([1, C_OUT], f32)
        nc.scalar.activation(u[:], ta[:], AF.Sin, bias=bias_zero[:], scale=0.5)
        v = sb.tile([1, C_OUT], f32)
        nc.scalar.activation(v[:], ta[:], AF.Sin, bias=bias_half_pi[:], scale=-0.5)
        u2 = sb.tile([1, C_OUT], f32)
        nc.scalar.activation(
            u2[:], u[:], AF.Square, bias=bias_zero[:], scale=math.sqrt(2.0)
        )
        nc.scalar.activation(
            csrow[:, 0:C_OUT], u2[:], AF.Copy, bias=1.0, scale=-1.0
        )
        act_raw(tdum[:], bias_eps[:], AF.Reciprocal, bias=1e-24)
        uv = sb.tile([1, C_OUT], f32)
        nc.vector.tensor_mul(uv[:], u[:], v[:])
        nc.vector.scalar_tensor_tensor(
            csrow[:, C_OUT : 2 * C_OUT], uv[:], 2.0, tsgn[:],
            op0=ALU.mult, op1=ALU.mult,
        )


        csmat = sb.tile([P, 2 * C_OUT], f32r)
        et = [sb.tile([P, P], f32r, name=f"et{k}", tag=f"et{k}") for k in range(4)]
        etn = [sb.tile([P, P], f32r, name=f"etn{k}", tag=f"etn{k}") for k in range(2)]
        gz = sb.tile([P, P], f32r)
        gx = sb.tile([P, P], f32r)
        invb = sb.tile([C_OUT, LP], f32)

        # PSUM pools in strict stack order: psA+psG (4 banks, outer, live
        # throughout), psB (qz/qx, 4 banks) closed before psC (out1) opens.
        with _ExitStack() as ps_stack:
            psA = ps_stack.enter_context(
                tc.tile_pool(name="psA", bufs=3, space="PSUM")
            )
            psG = ps_stack.enter_context(
                tc.tile_pool(name="psG", bufs=2, space="PSUM")
            )
            psB_cm = tc.tile_pool(name="psB", bufs=1, space="PSUM")
            psB = psB_cm.__enter__()

            # E^T chunks (PE order: these first — only need e128 + ident)
            for k in range(4):
                etps = psA.tile([P, P], f32, tag="eps")
                nc.tensor.transpose(
                    etps[:], e128[:, 128 * k : 128 * (k + 1)], ident[:]
                )
                nc.scalar.copy(et[k][:], etps[:])
            for i, k in enumerate((2, 3)):
                nc.vector.tensor_scalar_mul(etn[i][:], et[k][:], -1.0)

            ptr = sb.tile([P, LP], f32r)


            # GZ = F F^T - G G^T ; GX = F G^T + G F^T
            gzps = psG.tile([P, P], f32, tag="gram")
            nc.tensor.matmul(gzps[:], et[0][:], et[0][:], start=True, stop=False)
            nc.tensor.matmul(gzps[:], et[1][:], et[1][:], start=False, stop=False)
            nc.tensor.matmul(gzps[:], etn[0][:], et[2][:], start=False, stop=False)
            nc.tensor.matmul(gzps[:], etn[1][:], et[3][:], start=False, stop=True)

            gxps = psG.tile([P, P], f32, tag="gram")
            nc.tensor.matmul(gxps[:], et[0][:], et[2][:], start=True, stop=False)
            nc.tensor.matmul(gxps[:], et[1][:], et[3][:], start=False, stop=False)
            nc.tensor.matmul(gxps[:], et[2][:], et[0][:], start=False, stop=False)
            nc.tensor.matmul(gxps[:], et[3][:], et[1][:], start=False, stop=True)

            # ACT: squares (feed the 1/n2 chain) then gz/gx evacuations;
            # DVE row-reduces pipelined behind the squares
            nc.vector.tensor_copy(gz[:], gzps[:])
            nc.vector.tensor_copy(gx[:], gxps[:])

            # cos/sin broadcast to 128 partitions; evac on DVE (all of this is
            # pt-independent, so it runs while x is still streaming in)
            csb_ps = psA.tile([P, 2 * C_OUT], f32, tag="eps")
            nc.tensor.matmul(
                csb_ps[:], ones1[:].bitcast(f32r), csrow[:], start=True, stop=True
            )
            nc.vector.tensor_copy(csmat[:], csb_ps[:])

            # pt-dependent: fp32r round of pt (DVE) and squares (ACT),
            # per l-half so work starts as soon as the first half lands
            sq = sb.tile([P, LP], f32r)
            for h in range(2):
                s = slice(512 * h, 512 * (h + 1))
                nc.vector.tensor_copy(ptr[:, s], pt[:, s])
                nc.scalar.activation(sq[:, s], pt[:, s], AF.Square)

            # main quadratic forms interleaved with the n2/reciprocal chain
            invrow = sb.tile([1, LP], f32r)
            mzn = sb.tile([P, LP], f32r)
            mxn = sb.tile([P, LP], f32r)
            for h in range(2):
                s = slice(512 * h, 512 * (h + 1))
                qzh = psB.tile([P, 512], f32, name=f"qz{h}", tag="q", bufs=3)
                nc.tensor.matmul(qzh[:], gz[:], ptr[:, s], start=True, stop=True)
                qxh = psB.tile([P, 512], f32, name=f"qx{h}", tag="q", bufs=3)
                nc.tensor.matmul(qxh[:], gx[:], ptr[:, s], start=True, stop=True)
                n2row = psB.tile([1, 512], f32, name=f"n2row{h}", tag="n2r")
                nc.tensor.matmul(
                    n2row[:], onesc[:].bitcast(f32r), sq[:, s],
                    start=True, stop=True,
                )
                act_raw(invrow[0:1, s], n2row[:], AF.Reciprocal, bias=1e-24)
                nc.vector.tensor_mul(mzn[:, s], pt[:, s], qzh[:])
                nc.vector.tensor_mul(mxn[:, s], pt[:, s], qxh[:])


            psB_cm.__exit__(None, None, None)

            # channel combine: out1[c,l] = cos_c*qZ[l] + sin_c*qX[l]
            psC = ps_stack.enter_context(
                tc.tile_pool(name="psC", bufs=1, space="PSUM")
            )
            # broadcast 1/n2 to the 16 output channels via K=1 matmul
            invb_ps = psC.tile([C_OUT, LP], f32, tag="invb")
            for h in range(2):
                s = slice(512 * h, 512 * (h + 1))
                nc.tensor.matmul(
                    invb_ps[:, s], ones1[0:1, 0:C_OUT].bitcast(f32r),
                    invrow[0:1, s], start=True, stop=True,
                )
            for h in range(2):
                s = slice(512 * h, 512 * (h + 1))
                nc.scalar.copy(invb[:, s], invb_ps[:, s])
            outs = sb.tile([C_OUT, LP], f32)
            for h in range(2):
                s = slice(512 * h, 512 * (h + 1))
                out1 = psC.tile([C_OUT, 512], f32, name=f"out1_{h}", tag=f"o{h}")
                nc.tensor.matmul(
                    out1[:], csmat[:, 0:C_OUT], mzn[:, s],
                    start=True, stop=False,
                )
                nc.tensor.matmul(
                    out1[:], csmat[:, C_OUT : 2 * C_OUT], mxn[:, s],
                    start=False, stop=True,
                )
                # divide by ||p||^2 while evacuating PSUM, pipelined per half
                nc.vector.tensor_mul(outs[:, s], invb[:, s], out1[:])
                nc.sync.dma_start(out_ext[:, s], outs[:, s])


    nc.compile()
    return nc


# revision 26
# speedup vs baseline: 1.0282x; 1.0282x over previous
"""DenseQConv1D Trainium2 kernel.

Math: the reference computes, per output channel c and patch p (128-dim im2col
column of x, normalized):
    out[c,p] = sum_e sign(e) * (s_p^T (E @ R_c)[:128,:])_e^2
with R_c = kron of 9 RY(theta[c,q]) rotations and sign(e) = Z on the MSB qubit.
Because every RY factor is orthogonal and the measurement only touches qubit 0,
    R_c S R_c^T = kron([[cos t, sin t], [sin t, -cos t]], I_256),  t = theta[c,0]
so with E128 = E[:128,:], F = E128[:,:256], G = E128[:,256:]:
    GZ = F F^T - G G^T,  GX = F G^T + G F^T   (both 128x128, theta-independent)
    out[c,p] = (cos t_c * p^T GZ p + sin t_c * p^T GX p) / ||p||^2

Fast path: the entangle matrix built by the reference is a composition of CNOT
permutations, i.e. a permutation matrix that is linear over GF(2).  For any
such E, GZ is a +/-1 diagonal and GX restricted to the 128-dim patch subspace
is either an XOR-pairing or zero; for the CNOT ring used here GX == 0.  Then
    out[c,l] = cos t_c * (sum_j d_j pt[j,l]^2) / (sum_j pt[j,l]^2)
which needs no dense quadratic form at all.  The host folds theta and E into a
single [128, 48] stationary W = [d_j*cos_c | 0 | ones] and the device computes,
per l-chunk: sq = pt*pt (DVE, bf16), ps = W^T @ sq (PE: rows 0:16 give
cos_c*z_l, rows 32:48 give n2_l broadcast 16-wide), invb = 1/ps[32:48] (ACT,
which also bridges the 32-aligned PSUM partitions down to 0:16), and
out = ps[0:16]*invb (DVE).  The host verifies the GZ/GX structure numerically
and falls back to the fully generic kernel below for any other entangle input.

Sharding: batch dimension across the 8 cores (core b computes x[b]).
"""

import math
from contextlib import ExitStack as _ExitStack

import numpy as np

B = 8
C_IN = 16
C_OUT = 16
L = 1024
K = 8
L_OUT = L - K + 1  # 1017
LP = 1024  # padded patch count per core (cols 1017:1024 are dummy)
P = 128  # patch vector length = C_IN*K = partitions
XPAD = L + K  # padded x row length so every im2col window read is in-bounds

_CACHE = {}


def _build_nc_fast():
    import bass_rust as _br
    import concourse.bacc as bacc
    import concourse.mybir as mybir
    import concourse.tile as tile

    f32 = mybir.dt.float32
    bf16 = mybir.dt.bfloat16
    AF = mybir.ActivationFunctionType

    nc = bacc.Bacc("TRN2", target_bir_lowering=False, debug=False)

    def act_raw(out, in_, func, bias=0.0, scale=1.0):
        eng = nc.scalar
        ins = [
            eng.lower_ap(in_),
            mybir.ImmediateValue(dtype=mybir.dt.float32, value=bias),
            mybir.ImmediateValue(dtype=mybir.dt.float32, value=scale),
            mybir.ImmediateValue(dtype=mybir.dt.float32, value=0.0),
        ]
        return eng.add_instruction(
            mybir.InstActivation(
                name=nc.get_next_instruction_name(), func=func,
                ins=ins, outs=[eng.lower_ap(out)],
            )
        )

    xb_ext = nc.declare_dram_parameter("xb", [C_IN, XPAD], bf16, isOutput=False)
    wm_ext = nc.declare_dram_parameter("wm", [P, 3 * C_OUT], bf16, isOutput=False)
    out_ext = nc.declare_dram_parameter("out", [C_OUT, LP], f32, isOutput=True)

    with tile.TileContext(nc) as tc, tc.tile_pool(name="sb", bufs=1) as sb, \
            tc.tile_pool(name="ps", bufs=1, space="PSUM") as psp:
        # pt chunks on the sync ring first — they gate everything; uneven
        # chunks (big first, small last) so early chunks' compute overlaps
        # later transfers and the final chunk's serial tail is short
        # (measured best vs 2x512, 4x256, 1x1024, and dual-ring splits —
        # the first-chunk latency is HBM-latency-bound, and smaller chunks
        # pay extra ACTIVATE pipeline fills and DMA issue slots)
        chunks = [(0, 512), (512, 256), (768, 256)]
        pt = sb.tile([P, LP], bf16)
        for lo, n in chunks:
            apx = _br.AP(xb_ext, lo, [[1, K], [XPAD, C_IN], [1, n]])
            nc.sync.dma_start(pt[:, lo : lo + n], apx)

        # wm on the scalar ring, then the table-load trigger (first ACT op is
        # a Reciprocal so the single table set loads during the input DMA)
        wmt = sb.tile([P, 3 * C_OUT], bf16)
        nc.scalar.dma_start(wmt[:], wm_ext[:, :])
        tdum = sb.tile([1, 1], f32)
        nc.vector.memset(tdum[:], 1.0)
        tdum2 = sb.tile([1, 1], f32)
        act_raw(tdum2[:], tdum[:], AF.Reciprocal, bias=1e-24)

        sq = sb.tile([P, LP], bf16)
        # engine operand partition bases must be 32-aligned, so n2 lands at
        # PSUM partitions 32:48 (wm cols 32:48 are ones) and the ACT
        # reciprocal bridges it down to partitions 0:16 for the DVE multiply
        invb = sb.tile([C_OUT, LP], f32)
        outs = sb.tile([C_OUT, LP], f32)
        ph = []
        for i, (lo, n) in enumerate(chunks):
            s = slice(lo, lo + n)
            nc.vector.tensor_mul(sq[:, s], pt[:, s], pt[:, s])
            # full-bank tiles so chunks never share a PSUM bank
            ph.append(psp.tile([3 * C_OUT, 512], f32, name=f"ps{i}", tag=f"ps{i}"))
            nc.tensor.matmul(
                ph[i][:, 0:n], wmt[:], sq[:, s], start=True, stop=True
            )
        for i, (lo, n) in enumerate(chunks):
            s = slice(lo, lo + n)
            act_raw(
                invb[:, s], ph[i][2 * C_OUT : 3 * C_OUT, 0:n],
                AF.Reciprocal, bias=1e-24,
            )
        for i, (lo, n) in enumerate(chunks):
            s = slice(lo, lo + n)
            nc.vector.tensor_mul(outs[:, s], ph[i][0:C_OUT, 0:n], invb[:, s])
            # out chunks alternate across the two HWDGE rings
            eng = nc.sync if i % 2 == 0 else nc.scalar
            eng.dma_start(out_ext[:, s], outs[:, s])

    nc.compile()
    return nc


def _host_fold(x, theta, ent):
    """Fold theta and the entangle matrix into the fast-path inputs.

    Returns a list of per-core in_maps, or None if the entangle matrix does
    not have the (GZ diagonal, GX == 0) structure the fast kernel assumes.
    """
    import ml_dtypes

    E128 = ent[:P]
    F, G = E128[:, :256], E128[:, 256:]
    GZ = F @ F.T - G @ G.T
    GX = F @ G.T + G @ F.T
    dg = np.diag(GZ).copy()
    if np.abs(GZ - np.diag(dg)).max() > 1e-5 or np.abs(GX).max() > 1e-5:
        return None
    cos = np.cos(theta[:, 0].astype(np.float64))
    # device partition p = j*16+c maps to reference feature f = c*8+j
    p = np.arange(P)
    d_dev = dg[(p % C_IN) * K + p // C_IN]
    wm = np.zeros((P, 3 * C_OUT), np.float32)
    wm[:, :C_OUT] = d_dev[:, None] * cos[None, :]
    wm[:, 2 * C_OUT :] = 1.0
    wm_bf = np.ascontiguousarray(wm.astype(ml_dtypes.bfloat16))
    xpad = np.full((B, C_IN, XPAD), 1.0, np.float32)
    xpad[:, :, :L] = x
    xb = xpad.astype(ml_dtypes.bfloat16)
    return [
        {"xb": np.ascontiguousarray(xb[b]), "wm": wm_bf} for b in range(B)
    ]


def prepare(inputs):
    """Returns (nc, in_maps). Shared by kernel() and test.py."""
    x = np.ascontiguousarray(np.asarray(inputs["x"], dtype=np.float32))
    theta = np.ascontiguousarray(np.asarray(inputs["theta"], dtype=np.float32))
    ent = np.ascontiguousarray(
        np.asarray(inputs["entangle_matrix"], dtype=np.float32)
    )
    fold = _host_fold(x, theta, ent)
    if fold is not None:
        if "nc_fast" not in _CACHE:
            _CACHE["nc_fast"] = _build_nc_fast()
        return _CACHE["nc_fast"], fold
    if "nc" not in _CACHE:
        _CACHE["nc"] = _build_nc()
    in_maps = [
        {"x": np.ascontiguousarray(x[b]), "theta": theta, "entangle": ent}
        for b in range(B)
    ]
    return _CACHE["nc"], in_maps


def kernel(**inputs):
    from concourse.bass_utils import run_bass_kernel_spmd

    nc, in_maps = prepare(inputs)
    fast = nc is _CACHE.get("nc_fast")
    # fast path: |out[c,l]| = |cos(t_c) * z_l / n2_l| <= 1 mathematically
    # (|z| <= n2); anything far above that means a transient device glitch
    # -> retry (the bound needs E orthogonal, so fast path only)
    for _attempt in range(3):
        res = run_bass_kernel_spmd(nc, in_maps, core_ids=list(range(B)))
        out = np.stack(
            [res.results[b]["out"][:, :L_OUT] for b in range(B)], axis=0
        )
        ok = np.isfinite(out).all() and (not fast or np.abs(out).max() < 1.25)
        if ok:
            break
    return np.ascontiguousarray(out.astype(np.float32))


# ---------------------------------------------------------------------------
# Generic fallback: exact for any entangle matrix / theta (dense GZ/GX built
# on device; fp32r quadratic forms).  Used only when _host_fold rejects.
# ---------------------------------------------------------------------------

def _build_nc(dbg=False):
    import bass_rust as _br
    import concourse.bacc as bacc
    import concourse.mybir as mybir
    import concourse.tile as tile
    from concourse import masks

    f32 = mybir.dt.float32
    f32r = mybir.dt.float32r
    AF = mybir.ActivationFunctionType
    ALU = mybir.AluOpType

    def act_raw(out, in_, func, bias=0.0, scale=1.0):
        eng = nc.scalar
        ins = [
            eng.lower_ap(in_),
            mybir.ImmediateValue(dtype=mybir.dt.float32, value=bias),
            mybir.ImmediateValue(dtype=mybir.dt.float32, value=scale),
            mybir.ImmediateValue(dtype=mybir.dt.float32, value=0.0),
        ]
        return eng.add_instruction(
            mybir.InstActivation(
                name=nc.get_next_instruction_name(), func=func,
                ins=ins, outs=[eng.lower_ap(out)],
            )
        )
    nc = bacc.Bacc("TRN2", target_bir_lowering=False, debug=False)

    x_ext = nc.declare_dram_parameter("x", [C_IN, L], f32, isOutput=False)
    th_ext = nc.declare_dram_parameter("theta", [C_OUT, 9], f32, isOutput=False)
    e_ext = nc.declare_dram_parameter("entangle", [512, 512], f32, isOutput=False)
    out_ext = nc.declare_dram_parameter("out", [C_OUT, LP], f32, isOutput=True)

    with tile.TileContext(nc) as tc, tc.tile_pool(name="const", bufs=1) as const, \
            tc.tile_pool(name="sb", bufs=1) as sb, \
            tc.tile_pool(name="scr", bufs=2) as scrp:
        ident = const.tile([P, P], f32)
        masks.make_identity(nc, ident[:])
        ones1 = const.tile([1, P], f32)
        nc.vector.memset(ones1[:], 1.0)
        onesc = const.tile([P, 1], f32)
        nc.vector.memset(onesc[:], 1.0)

        # ---- loads: theta (tiny) first, then E halves, then x halves;
        # interleaved across the two HWDGE rings so E lands first ----
        th = sb.tile([1, C_OUT], f32)
        nc.sync.dma_start(th[:], th_ext[:, 0:1].rearrange("p o -> o p"))

        # E rows 0..127, row-permuted (c j) -> (j c); partition halves on the
        # two HWDGE rings (the gpsimd SWDGE path is ~26x slower here - avoid)
        e128 = sb.tile([P, 512], f32)
        e_p0 = _br.AP(e_ext, 0, [[512, 4], [512 * K, C_IN], [1, 512]])
        e_p1 = _br.AP(e_ext, 4 * 512, [[512, 4], [512 * K, C_IN], [1, 512]])
        nc.sync.dma_start(e128[0:64, :], e_p0)
        nc.scalar.dma_start(e128[64:P, :], e_p1)

        # PT[j*16+c, l] = x[c, l+j]: im2col via overlapping-window APs
        pt = sb.tile([P, LP], f32)
        # l-halves so the first 512 patch columns can be consumed while the
        # second half still streams; partition halves across the two rings
        for hl, (lo, ln) in enumerate([(0, 512), (512, L_OUT - 512)]):
            xw0 = _br.AP(x_ext, lo, [[1, 4], [L, C_IN], [1, ln]])
            xw1 = _br.AP(x_ext, 4 + lo, [[1, 4], [L, C_IN], [1, ln]])
            nc.sync.dma_start(pt[0:64, lo : lo + ln], xw0)
            nc.scalar.dma_start(pt[64:P, lo : lo + ln], xw1)
        nc.vector.memset(pt[:, L_OUT:LP], 1.0)

        # ACT uses exactly two PWP tables: trig_and_small (sin/square/copy)
        # then natural_log_exp_and_others (ln/exp/square/copy). Biases are DVE
        # memsets so the first ACT op is the Sin that loads the trig table;
        # a dummy Ln right after the trig block performs the single switch.
        bias_zero = const.tile([1, 1], f32)
        nc.vector.memset(bias_zero[:], 0.0)
        bias_half_pi = const.tile([1, 1], f32)
        nc.vector.memset(bias_half_pi[:], math.pi / 2.0)
        bias_eps = const.tile([1, 1], f32)
        nc.vector.memset(bias_eps[:], 1e-24)
        tdum = const.tile([1, 1], f32)

        # |t| and sign(t) on DVE (cheap, keeps ACT free)
        csrow = sb.tile([1, 2 * C_OUT], f32r)
        ta = sb.tile([1, C_OUT], f32)
        nc.vector.scalar_tensor_tensor(
            ta[:], th[:], -1.0, th[:], op0=ALU.mult, op1=ALU.max
        )
        tsgn = sb.tile([1, C_OUT], f32)
        nc.vector.tensor_scalar(tsgn[:], th[:], 0.0, None, op0=ALU.is_gt)
        nc.vector.tensor_scalar(
            tsgn[:], tsgn[:], 2.0, 1.0, op0=ALU.mult, op1=ALU.subtract
        )

        # trig: u = sin(|t|/2), v = cos(|t|/2); cos t = 1-2u^2,
        # sin t = sign(t)*2uv
        nc.scalar.activation(tdum[:], bias_zero[:], AF.Sin, bias=bias_zero[:])
        u = sb.tile([1, C_OUT], f32)
        nc.scalar.activation(u[:], ta[:], AF.Sin, bias=bias_zero[:], scale=0.5)
        v = sb.tile([1, C_OUT], f32)
        nc.scalar.activation(v[:], ta[:], AF.Sin, bias=bias_half_pi[:], scale=-0.5)
        u2 = sb.tile([1, C_OUT], f32)
        nc.scalar.activation(
            u2[:], u[:], AF.Square, bias=bias_zero[:], scale=math.sqrt(2.0)
        )
        nc.scalar.activation(
            csrow[:, 0:C_OUT], u2[:], AF.Copy, bias=1.0, scale=-1.0
        )
        act_raw(tdum[:], bias_eps[:], AF.Reciprocal, bias=1e-24)
        uv = sb.tile([1, C_OUT], f32)
        nc.vector.tensor_mul(uv[:], u[:], v[:])
        nc.vector.scalar_tensor_tensor(
            csrow[:, C_OUT : 2 * C_OUT], uv[:], 2.0, tsgn[:],
            op0=ALU.mult, op1=ALU.mult,
        )


        csmat = sb.tile([P, 2 * C_OUT], f32r)
        et = [sb.tile([P, P], f32r, name=f"et{k}", tag=f"et{k}") for k in range(4)]
        etn = [sb.tile([P, P], f32r, name=f"etn{k}", tag=f"etn{k}") for k in range(2)]
        gz = sb.tile([P, P], f32r)
        gx = sb.tile([P, P], f32r)
        invb = sb.tile([C_OUT, LP], f32)

        # PSUM pools in strict stack order: psA+psG (4 banks, outer, live
        # throughout), psB (qz/qx, 4 banks) closed before psC (out1) opens.
        with _ExitStack() as ps_stack:
            psA = ps_stack.enter_context(
                tc.tile_pool(name="psA", bufs=3, space="PSUM")
            )
            psG = ps_stack.enter_context(
                tc.tile_pool(name="psG", bufs=2, space="PSUM")
            )
            psB_cm = tc.tile_pool(name="psB", bufs=1, space="PSUM")
            psB = psB_cm.__enter__()

            # E^T chunks (PE order: these first — only need e128 + ident)
            for k in range(4):
                etps = psA.tile([P, P], f32, tag="eps")
                nc.tensor.transpose(
                    etps[:], e128[:, 128 * k : 128 * (k + 1)], ident[:]
                )
                nc.scalar.copy(et[k][:], etps[:])
            for i, k in enumerate((2, 3)):
                nc.vector.tensor_scalar_mul(etn[i][:], et[k][:], -1.0)

            ptr = sb.tile([P, LP], f32r)


            # GZ = F F^T - G G^T ; GX = F G^T + G F^T
            gzps = psG.tile([P, P], f32, tag="gram")
            nc.tensor.matmul(gzps[:], et[0][:], et[0][:], start=True, stop=False)
            nc.tensor.matmul(gzps[:], et[1][:], et[1][:], start=False, stop=False)
            nc.tensor.matmul(gzps[:], etn[0][:], et[2][:], start=False, stop=False)
            nc.tensor.matmul(gzps[:], etn[1][:], et[3][:], start=False, stop=True)

            gxps = psG.tile([P, P], f32, tag="gram")
            nc.tensor.matmul(gxps[:], et[0][:], et[2][:], start=True, stop=False)
            nc.tensor.matmul(gxps[:], et[1][:], et[3][:], start=False, stop=False)
            nc.tensor.matmul(gxps[:], et[2][:], et[0][:], start=False, stop=False)
            nc.tensor.matmul(gxps[:], et[3][:], et[1][:], start=False, stop=True)

            # ACT: squares (feed the 1/n2 chain) then gz/gx evacuations;
            # DVE row-reduces pipelined behind the squares
            nc.vector.tensor_copy(gz[:], gzps[:])
            nc.vector.tensor_copy(gx[:], gxps[:])

            # cos/sin broadcast to 128 partitions; evac on DVE (all of this is
            # pt-independent, so it runs while x is still streaming in)
            csb_ps = psA.tile([P, 2 * C_OUT], f32, tag="eps")
            nc.tensor.matmul(
                csb_ps[:], ones1[:].bitcast(f32r), csrow[:], start=True, stop=True
            )
            nc.vector.tensor_copy(csmat[:], csb_ps[:])

            # pt-dependent: fp32r round of pt (DVE) and squares (ACT),
            # per l-half so work starts as soon as the first half lands
            sq = sb.tile([P, LP], f32r)
            for h in range(2):
                s = slice(512 * h, 512 * (h + 1))
                nc.vector.tensor_copy(ptr[:, s], pt[:, s])
                nc.scalar.activation(sq[:, s], pt[:, s], AF.Square)

            # main quadratic forms interleaved with the n2/reciprocal chain
            invrow = sb.tile([1, LP], f32r)
            mzn = sb.tile([P, LP], f32r)
            mxn = sb.tile([P, LP], f32r)
            for h in range(2):
                s = slice(512 * h, 512 * (h + 1))
                qzh = psB.tile([P, 512], f32, name=f"qz{h}", tag="q", bufs=3)
                nc.tensor.matmul(qzh[:], gz[:], ptr[:, s], start=True, stop=True)
                qxh = psB.tile([P, 512], f32, name=f"qx{h}", tag="q", bufs=3)
                nc.tensor.matmul(qxh[:], gx[:], ptr[:, s], start=True, stop=True)
                n2row = psB.tile([1, 512], f32, name=f"n2row{h}", tag="n2r")
                nc.tensor.matmul(
                    n2row[:], onesc[:].bitcast(f32r), sq[:, s],
                    start=True, stop=True,
                )
                act_raw(invrow[0:1, s], n2row[:], AF.Reciprocal, bias=1e-24)
                nc.vector.tensor_mul(mzn[:, s], pt[:, s], qzh[:])
                nc.vector.tensor_mul(mxn[:, s], pt[:, s], qxh[:])


            psB_cm.__exit__(None, None, None)

            # channel combine: out1[c,l] = cos_c*qZ[l] + sin_c*qX[l]
            psC = ps_stack.enter_context(
                tc.tile_pool(name="psC", bufs=1, space="PSUM")
            )
            # broadcast 1/n2 to the 16 output channels via K=1 matmul
            invb_ps = psC.tile([C_OUT, LP], f32, tag="invb")
            for h in range(2):
                s = slice(512 * h, 512 * (h + 1))
                nc.tensor.matmul(
                    invb_ps[:, s], ones1[0:1, 0:C_OUT].bitcast(f32r),
                    invrow[0:1, s], start=True, stop=True,
                )
            for h in range(2):
                s = slice(512 * h, 512 * (h + 1))
                nc.scalar.copy(invb[:, s], invb_ps[:, s])
            outs = sb.tile([C_OUT, LP], f32)
            for h in range(2):
                s = slice(512 * h, 512 * (h + 1))
                out1 = psC.tile([C_OUT, 512], f32, name=f"out1_{h}", tag=f"o{h}")
                nc.tensor.matmul(
                    out1[:], csmat[:, 0:C_OUT], mzn[:, s],
                    start=True, stop=False,
                )
                nc.tensor.matmul(
                    out1[:], csmat[:, C_OUT : 2 * C_OUT], mxn[:, s],
                    start=False, stop=True,
                )
                # divide by ||p||^2 while evacuating PSUM, pipelined per half
                nc.vector.tensor_mul(outs[:, s], invb[:, s], out1[:])
                nc.sync.dma_start(out_ext[:, s], outs[:, s])


    nc.compile()
    return nc


# revision 27
# speedup vs baseline: 1.0451x; 1.0164x over previous
"""DenseQConv1D Trainium2 kernel.

Math: the reference computes, per output channel c and patch p (128-dim im2col
column of x, normalized):
    out[c,p] = sum_e sign(e) * (s_p^T (E @ R_c)[:128,:])_e^2
with R_c = kron of 9 RY(theta[c,q]) rotations and sign(e) = Z on the MSB qubit.
Because every RY factor is orthogonal and the measurement only touches qubit 0,
    R_c S R_c^T = kron([[cos t, sin t], [sin t, -cos t]], I_256),  t = theta[c,0]
so with E128 = E[:128,:], F = E128[:,:256], G = E128[:,256:]:
    GZ = F F^T - G G^T,  GX = F G^T + G F^T   (both 128x128, theta-independent)
    out[c,p] = (cos t_c * p^T GZ p + sin t_c * p^T GX p) / ||p||^2

Fast path: the entangle matrix built by the reference is a composition of CNOT
permutations, i.e. a permutation matrix that is linear over GF(2).  For any
such E, GZ is a +/-1 diagonal and GX restricted to the 128-dim patch subspace
is either an XOR-pairing or zero; for the CNOT ring used here GX == 0.  Then
    out[c,l] = cos t_c * (sum_j d_j pt[j,l]^2) / (sum_j pt[j,l]^2)
which needs no dense quadratic form at all.  The host folds theta and E into a
single [128, 48] stationary W = [d_j*cos_c | 0 | ones] and the device computes,
per l-chunk: sq = pt*pt (DVE, bf16), ps = W^T @ sq (PE: rows 0:16 give
cos_c*z_l, rows 32:48 give n2_l broadcast 16-wide), invb = 1/ps[32:48] (ACT,
which also bridges the 32-aligned PSUM partitions down to 0:16), and
out = ps[0:16]*invb (DVE).  The host verifies the GZ/GX structure numerically
and falls back to the fully generic kernel below for any other entangle input.

Sharding: batch dimension across the 8 cores (core b computes x[b]).
"""

import math
from contextlib import ExitStack as _ExitStack

import numpy as np

B = 8
C_IN = 16
C_OUT = 16
L = 1024
K = 8
L_OUT = L - K + 1  # 1017
LP = 1024  # padded patch count per core (cols 1017:1024 are dummy)
P = 128  # patch vector length = C_IN*K = partitions
XPAD = L + K  # padded x row length so every im2col window read is in-bounds

_CACHE = {}


def _build_nc_fast():
    import bass_rust as _br
    import concourse.bacc as bacc
    import concourse.mybir as mybir
    import concourse.tile as tile

    f32 = mybir.dt.float32
    bf16 = mybir.dt.bfloat16
    AF = mybir.ActivationFunctionType

    nc = bacc.Bacc("TRN2", target_bir_lowering=False, debug=False)

    def act_raw(out, in_, func, bias=0.0, scale=1.0):
        eng = nc.scalar
        ins = [
            eng.lower_ap(in_),
            mybir.ImmediateValue(dtype=mybir.dt.float32, value=bias),
            mybir.ImmediateValue(dtype=mybir.dt.float32, value=scale),
            mybir.ImmediateValue(dtype=mybir.dt.float32, value=0.0),
        ]
        return eng.add_instruction(
            mybir.InstActivation(
                name=nc.get_next_instruction_name(), func=func,
                ins=ins, outs=[eng.lower_ap(out)],
            )
        )

    xb_ext = nc.declare_dram_parameter("xb", [C_IN, XPAD], bf16, isOutput=False)
    wm_ext = nc.declare_dram_parameter("wm", [P, 3 * C_OUT], bf16, isOutput=False)
    out_ext = nc.declare_dram_parameter("out", [C_OUT, LP], f32, isOutput=True)

    with tile.TileContext(nc) as tc, tc.tile_pool(name="sb", bufs=1) as sb, \
            tc.tile_pool(name="ps", bufs=1, space="PSUM") as psp:
        # pt transfers on the sync ring first — they gate everything.  The
        # first compute chunk's DMA is split in two so its first doorbell
        # (and descriptor drain) happens ~0.5us earlier; compute still runs
        # as 3 chunks (measured best vs 2x512, 4x256, 1x1024, dual-ring —
        # smaller compute chunks pay extra ACTIVATE pipeline fills)
        chunks = [(0, 512), (512, 256), (768, 256)]
        pt = sb.tile([P, LP], bf16)
        for lo, n in ((0, 256), (256, 256), (512, 256), (768, 256)):
            apx = _br.AP(xb_ext, lo, [[1, K], [XPAD, C_IN], [1, n]])
            nc.sync.dma_start(pt[:, lo : lo + n], apx)

        # wm on the scalar ring, then the table-load trigger (first ACT op is
        # a Reciprocal so the single table set loads during the input DMA)
        wmt = sb.tile([P, 3 * C_OUT], bf16)
        nc.scalar.dma_start(wmt[:], wm_ext[:, :])
        tdum = sb.tile([1, 1], f32)
        nc.vector.memset(tdum[:], 1.0)
        tdum2 = sb.tile([1, 1], f32)
        act_raw(tdum2[:], tdum[:], AF.Reciprocal, bias=1e-24)

        sq = sb.tile([P, LP], bf16)
        # engine operand partition bases must be 32-aligned, so n2 lands at
        # PSUM partitions 32:48 (wm cols 32:48 are ones) and the ACT
        # reciprocal bridges it down to partitions 0:16 for the DVE multiply
        invb = sb.tile([C_OUT, LP], f32)
        outs = sb.tile([C_OUT, LP], f32)
        ph = []
        for i, (lo, n) in enumerate(chunks):
            s = slice(lo, lo + n)
            nc.vector.tensor_mul(sq[:, s], pt[:, s], pt[:, s])
            # full-bank tiles so chunks never share a PSUM bank
            ph.append(psp.tile([3 * C_OUT, 512], f32, name=f"ps{i}", tag=f"ps{i}"))
            nc.tensor.matmul(
                ph[i][:, 0:n], wmt[:], sq[:, s], start=True, stop=True
            )
        for i, (lo, n) in enumerate(chunks):
            s = slice(lo, lo + n)
            act_raw(
                invb[:, s], ph[i][2 * C_OUT : 3 * C_OUT, 0:n],
                AF.Reciprocal, bias=1e-24,
            )
        for i, (lo, n) in enumerate(chunks):
            s = slice(lo, lo + n)
            nc.vector.tensor_mul(outs[:, s], ph[i][0:C_OUT, 0:n], invb[:, s])
            # out chunks alternate across the two HWDGE rings
            eng = nc.sync if i % 2 == 0 else nc.scalar
            eng.dma_start(out_ext[:, s], outs[:, s])

    nc.compile()
    return nc


def _host_fold(x, theta, ent):
    """Fold theta and the entangle matrix into the fast-path inputs.

    Returns a list of per-core in_maps, or None if the entangle matrix does
    not have the (GZ diagonal, GX == 0) structure the fast kernel assumes.
    """
    import ml_dtypes

    E128 = ent[:P]
    F, G = E128[:, :256], E128[:, 256:]
    GZ = F @ F.T - G @ G.T
    GX = F @ G.T + G @ F.T
    dg = np.diag(GZ).copy()
    if np.abs(GZ - np.diag(dg)).max() > 1e-5 or np.abs(GX).max() > 1e-5:
        return None
    cos = np.cos(theta[:, 0].astype(np.float64))
    # device partition p = j*16+c maps to reference feature f = c*8+j
    p = np.arange(P)
    d_dev = dg[(p % C_IN) * K + p // C_IN]
    wm = np.zeros((P, 3 * C_OUT), np.float32)
    wm[:, :C_OUT] = d_dev[:, None] * cos[None, :]
    wm[:, 2 * C_OUT :] = 1.0
    wm_bf = np.ascontiguousarray(wm.astype(ml_dtypes.bfloat16))
    xpad = np.full((B, C_IN, XPAD), 1.0, np.float32)
    xpad[:, :, :L] = x
    xb = xpad.astype(ml_dtypes.bfloat16)
    return [
        {"xb": np.ascontiguousarray(xb[b]), "wm": wm_bf} for b in range(B)
    ]


def prepare(inputs):
    """Returns (nc, in_maps). Shared by kernel() and test.py."""
    x = np.ascontiguousarray(np.asarray(inputs["x"], dtype=np.float32))
    theta = np.ascontiguousarray(np.asarray(inputs["theta"], dtype=np.float32))
    ent = np.ascontiguousarray(
        np.asarray(inputs["entangle_matrix"], dtype=np.float32)
    )
    fold = _host_fold(x, theta, ent)
    if fold is not None:
        if "nc_fast" not in _CACHE:
            _CACHE["nc_fast"] = _build_nc_fast()
        return _CACHE["nc_fast"], fold
    if "nc" not in _CACHE:
        _CACHE["nc"] = _build_nc()
    in_maps = [
        {"x": np.ascontiguousarray(x[b]), "theta": theta, "entangle": ent}
        for b in range(B)
    ]
    return _CACHE["nc"], in_maps


def kernel(**inputs):
    from concourse.bass_utils import run_bass_kernel_spmd

    nc, in_maps = prepare(inputs)
    fast = nc is _CACHE.get("nc_fast")
    # fast path: |out[c,l]| = |cos(t_c) * z_l / n2_l| <= 1 mathematically
    # (|z| <= n2); anything far above that means a transient device glitch
    # -> retry (the bound needs E orthogonal, so fast path only)
    for _attempt in range(3):
        res = run_bass_kernel_spmd(nc, in_maps, core_ids=list(range(B)))
        out = np.stack(
            [res.results[b]["out"][:, :L_OUT] for b in range(B)], axis=0
        )
        ok = np.isfinite(out).all() and (not fast or np.abs(out).max() < 1.25)
        if ok:
            break
    return np.ascontiguousarray(out.astype(np.float32))


# ---------------------------------------------------------------------------
# Generic fallback: exact for any entangle matrix / theta (dense GZ/GX built
# on device; fp32r quadratic forms).  Used only when _host_fold rejects.
# ---------------------------------------------------------------------------

def _build_nc(dbg=False):
    import bass_rust as _br
    import concourse.bacc as bacc
    import concourse.mybir as mybir
    import concourse.tile as tile
    from concourse import masks

    f32 = mybir.dt.float32
    f32r = mybir.dt.float32r
    AF = mybir.ActivationFunctionType
    ALU = mybir.AluOpType

    def act_raw(out, in_, func, bias=0.0, scale=1.0):
        eng = nc.scalar
        ins = [
            eng.lower_ap(in_),
            mybir.ImmediateValue(dtype=mybir.dt.float32, value=bias),
            mybir.ImmediateValue(dtype=mybir.dt.float32, value=scale),
            mybir.ImmediateValue(dtype=mybir.dt.float32, value=0.0),
        ]
        return eng.add_instruction(
            mybir.InstActivation(
                name=nc.get_next_instruction_name(), func=func,
                ins=ins, outs=[eng.lower_ap(out)],
            )
        )
    nc = bacc.Bacc("TRN2", target_bir_lowering=False, debug=False)

    x_ext = nc.declare_dram_parameter("x", [C_IN, L], f32, isOutput=False)
    th_ext = nc.declare_dram_parameter("theta", [C_OUT, 9], f32, isOutput=False)
    e_ext = nc.declare_dram_parameter("entangle", [512, 512], f32, isOutput=False)
    out_ext = nc.declare_dram_parameter("out", [C_OUT, LP], f32, isOutput=True)

    with tile.TileContext(nc) as tc, tc.tile_pool(name="const", bufs=1) as const, \
            tc.tile_pool(name="sb", bufs=1) as sb, \
            tc.tile_pool(name="scr", bufs=2) as scrp:
        ident = const.tile([P, P], f32)
        masks.make_identity(nc, ident[:])
        ones1 = const.tile([1, P], f32)
        nc.vector.memset(ones1[:], 1.0)
        onesc = const.tile([P, 1], f32)
        nc.vector.memset(onesc[:], 1.0)

        # ---- loads: theta (tiny) first, then E halves, then x halves;
        # interleaved across the two HWDGE rings so E lands first ----
        th = sb.tile([1, C_OUT], f32)
        nc.sync.dma_start(th[:], th_ext[:, 0:1].rearrange("p o -> o p"))

        # E rows 0..127, row-permuted (c j) -> (j c); partition halves on the
        # two HWDGE rings (the gpsimd SWDGE path is ~26x slower here - avoid)
        e128 = sb.tile([P, 512], f32)
        e_p0 = _br.AP(e_ext, 0, [[512, 4], [512 * K, C_IN], [1, 512]])
        e_p1 = _br.AP(e_ext, 4 * 512, [[512, 4], [512 * K, C_IN], [1, 512]])
        nc.sync.dma_start(e128[0:64, :], e_p0)
        nc.scalar.dma_start(e128[64:P, :], e_p1)

        # PT[j*16+c, l] = x[c, l+j]: im2col via overlapping-window APs
        pt = sb.tile([P, LP], f32)
        # l-halves so the first 512 patch columns can be consumed while the
        # second half still streams; partition halves across the two rings
        for hl, (lo, ln) in enumerate([(0, 512), (512, L_OUT - 512)]):
            xw0 = _br.AP(x_ext, lo, [[1, 4], [L, C_IN], [1, ln]])
            xw1 = _br.AP(x_ext, 4 + lo, [[1, 4], [L, C_IN], [1, ln]])
            nc.sync.dma_start(pt[0:64, lo : lo + ln], xw0)
            nc.scalar.dma_start(pt[64:P, lo : lo + ln], xw1)
        nc.vector.memset(pt[:, L_OUT:LP], 1.0)

        # ACT uses exactly two PWP tables: trig_and_small (sin/square/copy)
        # then natural_log_exp_and_others (ln/exp/square/copy). Biases are DVE
        # memsets so the first ACT op is the Sin that loads the trig table;
        # a dummy Ln right after the trig block performs the single switch.
        bias_zero = const.tile([1, 1], f32)
        nc.vector.memset(bias_zero[:], 0.0)
        bias_half_pi = const.tile([1, 1], f32)
        nc.vector.memset(bias_half_pi[:], math.pi / 2.0)
        bias_eps = const.tile([1, 1], f32)
        nc.vector.memset(bias_eps[:], 1e-24)
        tdum = const.tile([1, 1], f32)

        # |t| and sign(t) on DVE (cheap, keeps ACT free)
        csrow = sb.tile([1, 2 * C_OUT], f32r)
        ta = sb.tile([1, C_OUT], f32)
        nc.vector.scalar_tensor_tensor(
            ta[:], th[:], -1.0, th[:], op0=ALU.mult, op1=ALU.max
        )
        tsgn = sb.tile([1, C_OUT], f32)
        nc.vector.tensor_scalar(tsgn[:], th[:], 0.0, None, op0=ALU.is_gt)
        nc.vector.tensor_scalar(
            tsgn[:], tsgn[:], 2.0, 1.0, op0=ALU.mult, op1=ALU.subtract
        )

        # trig: u = sin(|t|/2), v = cos(|t|/2); cos t = 1-2u^2,
        # sin t = sign(t)*2uv
        nc.scalar.activation(tdum[:], bias_zero[:], AF.Sin, bias=bias_zero[:])
        u = sb.tile([1, C_OUT], f32)
        nc.scalar.activation(u[:], ta[:], AF.Sin, bias=bias_zero[:], scale=0.5)
        v = sb.tile([1, C_OUT], f32)
        nc.scalar.activation(v[:], ta[:], AF.Sin, bias=bias_half_pi[:], scale=-0.5)
        u2 = sb.tile([1, C_OUT], f32)
        nc.scalar.activation(
            u2[:], u[:], AF.Square, bias=bias_zero[:], scale=math.sqrt(2.0)
        )
        nc.scalar.activation(
            csrow[:, 0:C_OUT], u2[:], AF.Copy, bias=1.0, scale=-1.0
        )
        act_raw(tdum[:], bias_eps[:], AF.Reciprocal, bias=1e-24)
        uv = sb.tile([1, C_OUT], f32)
        nc.vector.tensor_mul(uv[:], u[:], v[:])
        nc.vector.scalar_tensor_tensor(
            csrow[:, C_OUT : 2 * C_OUT], uv[:], 2.0, tsgn[:],
            op0=ALU.mult, op1=ALU.mult,
        )


        csmat = sb.tile([P, 2 * C_OUT], f32r)
        et = [sb.tile([P, P], f32r, name=f"et{k}", tag=f"et{k}") for k in range(4)]
        etn = [sb.tile([P, P], f32r, name=f"etn{k}", tag=f"etn{k}") for k in range(2)]
        gz = sb.tile([P, P], f32r)
        gx = sb.tile([P, P], f32r)
        invb = sb.tile([C_OUT, LP], f32)

        # PSUM pools in strict stack order: psA+psG (4 banks, outer, live
        # throughout), psB (qz/qx, 4 banks) closed before psC (out1) opens.
        with _ExitStack() as ps_stack:
            psA = ps_stack.enter_context(
                tc.tile_pool(name="psA", bufs=3, space="PSUM")
            )
            psG = ps_stack.enter_context(
                tc.tile_pool(name="psG", bufs=2, space="PSUM")
            )
            psB_cm = tc.tile_pool(name="psB", bufs=1, space="PSUM")
            psB = psB_cm.__enter__()

            # E^T chunks (PE order: these first — only need e128 + ident)
            for k in range(4):
                etps = psA.tile([P, P], f32, tag="eps")
                nc.tensor.transpose(
                    etps[:], e128[:, 128 * k : 128 * (k + 1)], ident[:]
                )
                nc.scalar.copy(et[k][:], etps[:])
            for i, k in enumerate((2, 3)):
                nc.vector.tensor_scalar_mul(etn[i][:], et[k][:], -1.0)

            ptr = sb.tile([P, LP], f32r)


            # GZ = F F^T - G G^T ; GX = F G^T + G F^T
            gzps = psG.tile([P, P], f32, tag="gram")
            nc.tensor.matmul(gzps[:], et[0][:], et[0][:], start=True, stop=False)
            nc.tensor.matmul(gzps[:], et[1][:], et[1][:], start=False, stop=False)
            nc.tensor.matmul(gzps[:], etn[0][:], et[2][:], start=False, stop=False)
            nc.tensor.matmul(gzps[:], etn[1][:], et[3][:], start=False, stop=True)

            gxps = psG.tile([P, P], f32, tag="gram")
            nc.tensor.matmul(gxps[:], et[0][:], et[2][:], start=True, stop=False)
            nc.tensor.matmul(gxps[:], et[1][:], et[3][:], start=False, stop=False)
            nc.tensor.matmul(gxps[:], et[2][:], et[0][:], start=False, stop=False)
            nc.tensor.matmul(gxps[:], et[3][:], et[1][:], start=False, stop=True)

            # ACT: squares (feed the 1/n2 chain) then gz/gx evacuations;
            # DVE row-reduces pipelined behind the squares
            nc.vector.tensor_copy(gz[:], gzps[:])
            nc.vector.tensor_copy(gx[:], gxps[:])

            # cos/sin broadcast to 128 partitions; evac on DVE (all of this is
            # pt-independent, so it runs while x is still streaming in)
            csb_ps = psA.tile([P, 2 * C_OUT], f32, tag="eps")
            nc.tensor.matmul(
                csb_ps[:], ones1[:].bitcast(f32r), csrow[:], start=True, stop=True
            )
            nc.vector.tensor_copy(csmat[:], csb_ps[:])

            # pt-dependent: fp32r round of pt (DVE) and squares (ACT),
            # per l-half so work starts as soon as the first half lands
            sq = sb.tile([P, LP], f32r)
            for h in range(2):
                s = slice(512 * h, 512 * (h + 1))
                nc.vector.tensor_copy(ptr[:, s], pt[:, s])
                nc.scalar.activation(sq[:, s], pt[:, s], AF.Square)

            # main quadratic forms interleaved with the n2/reciprocal chain
            invrow = sb.tile([1, LP], f32r)
            mzn = sb.tile([P, LP], f32r)
            mxn = sb.tile([P, LP], f32r)
            for h in range(2):
                s = slice(512 * h, 512 * (h + 1))
                qzh = psB.tile([P, 512], f32, name=f"qz{h}", tag="q", bufs=3)
                nc.tensor.matmul(qzh[:], gz[:], ptr[:, s], start=True, stop=True)
                qxh = psB.tile([P, 512], f32, name=f"qx{h}", tag="q", bufs=3)
                nc.tensor.matmul(qxh[:], gx[:], ptr[:, s], start=True, stop=True)
                n2row = psB.tile([1, 512], f32, name=f"n2row{h}", tag="n2r")
                nc.tensor.matmul(
                    n2row[:], onesc[:].bitcast(f32r), sq[:, s],
                    start=True, stop=True,
                )
                act_raw(invrow[0:1, s], n2row[:], AF.Reciprocal, bias=1e-24)
                nc.vector.tensor_mul(mzn[:, s], pt[:, s], qzh[:])
                nc.vector.tensor_mul(mxn[:, s], pt[:, s], qxh[:])


            psB_cm.__exit__(None, None, None)

            # channel combine: out1[c,l] = cos_c*qZ[l] + sin_c*qX[l]
            psC = ps_stack.enter_context(
                tc.tile_pool(name="psC", bufs=1, space="PSUM")
            )
            # broadcast 1/n2 to the 16 output channels via K=1 matmul
            invb_ps = psC.tile([C_OUT, LP], f32, tag="invb")
            for h in range(2):
                s = slice(512 * h, 512 * (h + 1))
                nc.tensor.matmul(
                    invb_ps[:, s], ones1[0:1, 0:C_OUT].bitcast(f32r),
                    invrow[0:1, s], start=True, stop=True,
                )
            for h in range(2):
                s = slice(512 * h, 512 * (h + 1))
                nc.scalar.copy(invb[:, s], invb_ps[:, s])
            outs = sb.tile([C_OUT, LP], f32)
            for h in range(2):
                s = slice(512 * h, 512 * (h + 1))
                out1 = psC.tile([C_OUT, 512], f32, name=f"out1_{h}", tag=f"o{h}")
                nc.tensor.matmul(
                    out1[:], csmat[:, 0:C_OUT], mzn[:, s],
                    start=True, stop=False,
                )
                nc.tensor.matmul(
                    out1[:], csmat[:, C_OUT : 2 * C_OUT], mxn[:, s],
                    start=False, stop=True,
                )
                # divide by ||p||^2 while evacuating PSUM, pipelined per half
                nc.vector.tensor_mul(outs[:, s], invb[:, s], out1[:])
                nc.sync.dma_start(out_ext[:, s], outs[:, s])


    nc.compile()
    return nc
